# revision 12
# baseline (speedup 1.0000x reference)
"""DelayAudio Trainium2 kernel.

Per-channel delay line with SOS/EOS/PAD masking:
  out[b, c, t] = SOS                       for t <  start_c           (start_c = c + offset)
               = audio[b, c, t - start_c]  for start_c <= t < start_c + len_b
               = EOS                       for start_c + len_b <= t < len_b + C
               = PAD                       for t >= len_b + C
Second output: audio_len + C.

Strategy: pure data-parallel over batch (8 examples per NeuronCore, 8 cores).
On-device dtype is int16 (values fit in [0, 1026]), which halves HBM traffic
vs int32.  The host uploads each core's shard in "delayed layout": rows are
(channel, batch) = 64 rows of width T+16, where row r already carries its
per-channel shift and SOS head (P[r, i] = SOS for i < start_c, else
audio[r, i - start_c]).  That choice of upload layout makes every device-side
load/store ONE dense 3D affine DMA over all 128 partitions.

The dynamic (audio_len-dependent) masking runs on device:
  mask = Relu(ramp + (1 - t1loc))     on ScalarE  (nonzero <=> t >= len + start_c)
  tail = (ramp >= t2loc) + 1025      on VectorE  (1025=EOS before len+C, 1026=PAD)
  copy_predicated(G, mask, tail)     on VectorE
with per-(row, tile) local thresholds precomputed on the host from audio_len
into tiny [128, 5] f32 tensors.  Loads issue on the SP HWDGE ring, stores on
the ACT HWDGE ring so the two directions pipeline independently.
"""

import numpy as np

_BASS_PATHS = ["/opt/trn_rl_repo", "/root/.axon_site/_ro/trn_rl_repo"]


def _ensure_import_paths():
    import sys

    try:
        import concourse.bass  # noqa: F401

        return
    except ImportError:
        pass
    for p in _BASS_PATHS:
        if p not in sys.path:
            sys.path.insert(0, p)
    import concourse.bass  # noqa: F401


# Problem constants (hardcoded per harness contract).
B = 64
C = 8
T = 32768
TOUT = T + C
N_CORES = 8
B_SHARD = B // N_CORES  # 8 examples per core
ROWS = B_SHARD * C      # 64 rows per core, ordered (c, b)
SOS = 1024
EOS = 1025
PAD = 1026

PADW = T + 16           # padded width of the host-uploaded delayed layout
TILE_W = 4096
N_ITERS = 4                   # main loop iterations; each covers 2 tiles (halves)
HALF_SPAN = N_ITERS * TILE_W  # 16384; half h covers [h*16384, (h+1)*16384)
TAIL_W = TOUT - 2 * HALF_SPAN  # 8 trailing columns

_CACHE: dict = {}
LAST_RESULTS = None  # BassKernelResults of the most recent run (for test harness)


def _build(offset: int, repeats: int = 1, mode: str = "full", dma_cfg: str = "sync_scalar"):
    """Build + finalize the (single, shared across cores) Bass program.

    repeats > 1 emits the whole pipeline body that many times (same I/O) —
    used only for timing (per-execution time from deltas between variants).
    mode: "full", "dma" (no compute), or "compute" (no audio/output DMAs).
    """
    import concourse.bass as bass
    import concourse.tile as tile
    from concourse import bacc, mybir

    AF = mybir.ActivationFunctionType
    OP = mybir.AluOpType
    i16 = mybir.dt.int16
    f32 = mybir.dt.float32

    nc = bacc.Bacc()
    audio16 = nc.declare_dram_parameter("audio16", [ROWS, PADW], i16, isOutput=False)
    thr2 = nc.declare_dram_parameter("thr2", [128, N_ITERS + 1], f32, isOutput=False)
    bias1 = nc.declare_dram_parameter("bias1", [128, N_ITERS + 1], f32, isOutput=False)
    out16 = nc.declare_dram_parameter("out16", [ROWS, TOUT], i16, isOutput=True)

    with_dma = mode in ("full", "dma")
    with_compute = mode in ("full", "compute")

    def load_eng(k):
        return {
            "sync_scalar": [nc.sync],
            "sync_only": [nc.sync],
            "swdge": [nc.gpsimd],
            "split4": [nc.sync, nc.scalar],
            "alt": [nc.sync, nc.scalar][k % 2:][:1],
        }[dma_cfg]

    def store_eng(k):
        return {
            "sync_scalar": [nc.scalar],
            "sync_only": [nc.sync],
            "swdge": [nc.gpsimd],
            "split4": [nc.gpsimd, nc.gpsimd],
            "alt": [nc.scalar, nc.sync][k % 2:][:1],
        }[dma_cfg]

    with tile.TileContext(nc) as tc:
        with (
            tc.tile_pool(name="consts", bufs=1) as consts,
            tc.tile_pool(name="gpool", bufs=4) as gpool,
            tc.tile_pool(name="mpool", bufs=3) as mpool,
            tc.tile_pool(name="tpool", bufs=3) as tpool,
        ):
            ramp = consts.tile([128, TILE_W], i16)
            nc.gpsimd.iota(ramp[:], pattern=[[1, TILE_W]], base=0, channel_multiplier=0)
            thr2_sb = consts.tile([128, N_ITERS + 1], f32)
            nc.sync.dma_start(thr2_sb[:], thr2[:])
            bias1_sb = consts.tile([128, N_ITERS + 1], f32)
            nc.sync.dma_start(bias1_sb[:], bias1[:])

            for _rep in range(repeats):
                for k in range(N_ITERS):
                    g = None
                    if with_dma:
                        g = gpool.tile([128, TILE_W], i16, tag="g")
                        engs = load_eng(k)
                        if len(engs) == 1:
                            src = bass.AP(
                                audio16,
                                k * TILE_W,
                                [[HALF_SPAN, 2], [PADW, ROWS], [1, TILE_W]],
                            )
                            engs[0].dma_start(g[:, :], src)
                        else:
                            for h, eng in enumerate(engs):
                                src = bass.AP(
                                    audio16,
                                    h * HALF_SPAN + k * TILE_W,
                                    [[PADW, ROWS], [1, TILE_W]],
                                )
                                eng.dma_start(g[h * ROWS:(h + 1) * ROWS, :], src)

                    if with_compute:
                        m = mpool.tile([128, TILE_W], i16, tag="m")
                        nc.scalar.activation(
                            m[:], ramp[:], AF.Relu, bias=bias1_sb[:, k:k + 1], scale=1.0
                        )
                        t = tpool.tile([128, TILE_W], i16, tag="t")
                        nc.vector.tensor_scalar(
                            t[:], ramp[:], thr2_sb[:, k:k + 1], 1025, OP.is_ge, OP.add
                        )
                        if g is None:
                            g = gpool.tile([128, TILE_W], i16, tag="g")
                        nc.vector.copy_predicated(g[:], m[:], t[:])

                    if with_dma:
                        engs = store_eng(k)
                        if len(engs) == 1:
                            dst = bass.AP(
                                out16,
                                k * TILE_W,
                                [[HALF_SPAN, 2], [TOUT, ROWS], [1, TILE_W]],
                            )
                            engs[0].dma_start(dst, g[:, :])
                        else:
                            for h, eng in enumerate(engs):
                                dst = bass.AP(
                                    out16,
                                    h * HALF_SPAN + k * TILE_W,
                                    [[TOUT, ROWS], [1, TILE_W]],
                                )
                                eng.dma_start(dst, g[h * ROWS:(h + 1) * ROWS, :])

                # Trailing TAIL_W columns at t0 = 2*HALF_SPAN = 32768.
                if mode != "full":
                    continue
                g8 = gpool.tile([ROWS, TAIL_W], i16, tag="g8")
                src = bass.AP(audio16, 2 * HALF_SPAN, [[PADW, ROWS], [1, TAIL_W]])
                nc.sync.dma_start(g8[:, :], src)
                m8 = mpool.tile([ROWS, TAIL_W], i16, tag="m8")
                nc.scalar.activation(
                    m8[:], ramp[0:ROWS, 0:TAIL_W], AF.Relu,
                    bias=bias1_sb[0:ROWS, N_ITERS:N_ITERS + 1], scale=1.0,
                )
                t8 = tpool.tile([ROWS, TAIL_W], i16, tag="t8")
                nc.vector.tensor_scalar(
                    t8[:], ramp[0:ROWS, 0:TAIL_W],
                    thr2_sb[0:ROWS, N_ITERS:N_ITERS + 1], 1025, OP.is_ge, OP.add,
                )
                nc.vector.copy_predicated(g8[:, :], m8[:], t8[:])
                dst = bass.AP(out16, 2 * HALF_SPAN, [[TOUT, ROWS], [1, TAIL_W]])
                nc.scalar.dma_start(dst, g8[:, :])

    nc.finalize()
    return nc


def _get_program(offset: int):
    import os
    cfg = os.environ.get("KERNEL_DMA_CFG", "sync_scalar")
    key = ("prog", offset, cfg)
    if key not in _CACHE:
        _CACHE[key] = _build(offset, dma_cfg=cfg)
    return _CACHE[key]


def _thresholds(lens: np.ndarray, offset: int):
    """Per-(partition, iteration) local thresholds for one core.

    lens: [B_SHARD] int array of this core's audio lengths.
    Returns (thr2 [128, 5] float32, bias1 [128, 5] float32).
    """
    p = np.arange(128)
    h = p // ROWS
    r = p % ROWS
    c = r // B_SHARD
    b = r % B_SHARD
    ln = lens[b]
    t1 = ln + c + offset          # first non-gathered position
    t2 = ln + C                   # first PAD position
    t0 = np.empty((128, N_ITERS + 1), dtype=np.int64)
    for k in range(N_ITERS):
        t0[:, k] = h * HALF_SPAN + k * TILE_W
    t0[:, N_ITERS] = 2 * HALF_SPAN
    thr2 = (t2[:, None] - t0).astype(np.float32)
    bias1 = np.minimum(1 - (t1[:, None] - t0), 8193).astype(np.float32)
    return thr2, bias1


def _make_in_maps(audio, audio_len, offset):
    in_maps = []
    for i in range(N_CORES):
        shard = audio[i * B_SHARD:(i + 1) * B_SHARD]            # [8, 8, T]
        delayed = np.empty((C, B_SHARD, PADW), dtype=np.int16)  # rows (c, b)
        for c in range(C):
            d = c + offset
            delayed[c, :, :d] = SOS
            delayed[c, :, d:d + T] = shard[:, c, :]
            delayed[c, :, d + T:] = 0
        lens = audio_len[i * B_SHARD:(i + 1) * B_SHARD].astype(np.int64)
        thr2, bias1 = _thresholds(lens, offset)
        in_maps.append({
            "audio16": delayed.reshape(ROWS, PADW),
            "thr2": thr2,
            "bias1": bias1,
        })
    return in_maps


def kernel(audio, audio_len, target=0, **_unused):
    global LAST_RESULTS
    _ensure_import_paths()
    from concourse.bass_utils import run_bass_kernel_spmd

    audio = np.asarray(audio)
    audio_len = np.asarray(audio_len)
    offset = 0 if target else 1

    nc = _get_program(offset)
    in_maps = _make_in_maps(audio, audio_len, offset)

    res = run_bass_kernel_spmd(nc, in_maps, list(range(N_CORES)))
    LAST_RESULTS = res

    parts = []
    for i in range(N_CORES):
        o = np.asarray(res.results[i]["out16"]).reshape(C, B_SHARD, TOUT)
        parts.append(o.transpose(1, 0, 2))                     # [8, 8, TOUT]
    out = np.concatenate(parts, axis=0).astype(audio.dtype)
    return out, (audio_len + C).astype(audio_len.dtype)


# revision 13
# speedup vs baseline: 4.8326x; 4.8326x over previous
"""DelayAudio Trainium2 kernel.

Per-channel delay line with SOS/EOS/PAD masking:
  out[b, c, t] = SOS                       for t <  start_c           (start_c = c + offset)
               = audio[b, c, t - start_c]  for start_c <= t < start_c + len_b
               = EOS                       for start_c + len_b <= t < len_b + C
               = PAD                       for t >= len_b + C
Second output: audio_len + C.

Strategy: pure data-parallel over batch (8 examples per NeuronCore, 8 cores).
On-device dtype is int16 (values fit in [0, 1026]), which halves HBM traffic
vs int32.  The host uploads each core's shard in "delayed layout": rows are
(channel, batch) = 64 rows of width T+16, where row r already carries its
per-channel shift and SOS head (P[r, i] = SOS for i < start_c, else
audio[r, i - start_c]).  That choice of upload layout makes every device-side
load/store ONE dense 3D affine DMA over all 128 partitions.

The dynamic (audio_len-dependent) masking runs on device:
  mask = Relu(ramp + (1 - t1loc))     on ScalarE  (nonzero <=> t >= len + start_c)
  tail = (ramp >= t2loc) + 1025      on VectorE  (1025=EOS before len+C, 1026=PAD)
  copy_predicated(G, mask, tail)     on VectorE
with per-(row, tile) local thresholds precomputed on the host from audio_len
into tiny [128, 5] f32 tensors.  Loads issue on the SP HWDGE ring, stores on
the ACT HWDGE ring so the two directions pipeline independently.
"""

import numpy as np

_BASS_PATHS = ["/opt/trn_rl_repo", "/root/.axon_site/_ro/trn_rl_repo"]


def _ensure_import_paths():
    import sys

    try:
        import concourse.bass  # noqa: F401

        return
    except ImportError:
        pass
    for p in _BASS_PATHS:
        if p not in sys.path:
            sys.path.insert(0, p)
    import concourse.bass  # noqa: F401


# Problem constants (hardcoded per harness contract).
B = 64
C = 8
T = 32768
TOUT = T + C
N_CORES = 8
B_SHARD = B // N_CORES  # 8 examples per core
ROWS = B_SHARD * C      # 64 rows per core, ordered (c, b)
SOS = 1024
EOS = 1025
PAD = 1026

PADW = T + 16           # padded width of the host-uploaded delayed layout
TILE_W = 4096
N_ITERS = 4                   # main loop iterations; each covers 2 tiles (halves)
HALF_SPAN = N_ITERS * TILE_W  # 16384; half h covers [h*16384, (h+1)*16384)
TAIL_W = TOUT - 2 * HALF_SPAN  # 8 trailing columns

_CACHE: dict = {}
LAST_RESULTS = None  # BassKernelResults of the most recent run (for test harness)


def _build(offset: int, repeats: int = 1, mode: str = "full", dma_cfg: str = "sync_scalar"):
    """Build + finalize the (single, shared across cores) Bass program.

    repeats > 1 emits the whole pipeline body that many times (same I/O) —
    used only for timing (per-execution time from deltas between variants).
    mode: "full", "dma" (no compute), or "compute" (no audio/output DMAs).
    """
    import concourse.bass as bass
    import concourse.tile as tile
    from concourse import bacc, mybir

    AF = mybir.ActivationFunctionType
    OP = mybir.AluOpType
    i16 = mybir.dt.int16
    f32 = mybir.dt.float32

    nc = bacc.Bacc()
    audio16 = nc.declare_dram_parameter("audio16", [ROWS, PADW], i16, isOutput=False)
    thr2 = nc.declare_dram_parameter("thr2", [128, N_ITERS + 1], f32, isOutput=False)
    bias1 = nc.declare_dram_parameter("bias1", [128, N_ITERS + 1], f32, isOutput=False)
    out16 = nc.declare_dram_parameter("out16", [ROWS, TOUT], i16, isOutput=True)

    with_dma = mode in ("full", "dma")
    with_compute = mode in ("full", "compute")

    ALL3 = [nc.sync, nc.scalar, nc.gpsimd]

    def load_eng(k):
        return {
            "sync_scalar": [nc.sync],
            "sync_only": [nc.sync],
            "swdge": [nc.gpsimd],
            "split4": [nc.sync, nc.scalar],
            "alt": [ALL3[k % 2]],
            "tri": [ALL3[k % 3]],
            "rr64": [ALL3[(2 * k) % 3], ALL3[(2 * k + 1) % 3]],
            "hw64": [nc.sync, nc.sync],
        }[dma_cfg]

    def store_eng(k):
        return {
            "sync_scalar": [nc.scalar],
            "sync_only": [nc.sync],
            "swdge": [nc.gpsimd],
            "split4": [nc.gpsimd, nc.gpsimd],
            "alt": [ALL3[(k + 1) % 2]],
            "tri": [ALL3[(k + 1) % 3]],
            "rr64": [ALL3[(2 * k + 2) % 3], ALL3[(2 * k) % 3]],
            "hw64": [nc.scalar, nc.scalar],
        }[dma_cfg]

    with tile.TileContext(nc) as tc:
        with (
            tc.tile_pool(name="consts", bufs=1) as consts,
            tc.tile_pool(name="gpool", bufs=4) as gpool,
            tc.tile_pool(name="mpool", bufs=3) as mpool,
            tc.tile_pool(name="tpool", bufs=3) as tpool,
        ):
            ramp = consts.tile([128, TILE_W], i16)
            nc.gpsimd.iota(ramp[:], pattern=[[1, TILE_W]], base=0, channel_multiplier=0)
            thr2_sb = consts.tile([128, N_ITERS + 1], f32)
            nc.sync.dma_start(thr2_sb[:], thr2[:])
            bias1_sb = consts.tile([128, N_ITERS + 1], f32)
            nc.sync.dma_start(bias1_sb[:], bias1[:])

            for _rep in range(repeats):
                for k in range(N_ITERS):
                    g = None
                    if with_dma:
                        g = gpool.tile([128, TILE_W], i16, tag="g")
                        engs = load_eng(k)
                        if len(engs) == 1:
                            src = bass.AP(
                                audio16,
                                k * TILE_W,
                                [[HALF_SPAN, 2], [PADW, ROWS], [1, TILE_W]],
                            )
                            engs[0].dma_start(g[:, :], src)
                        else:
                            for h, eng in enumerate(engs):
                                src = bass.AP(
                                    audio16,
                                    h * HALF_SPAN + k * TILE_W,
                                    [[PADW, ROWS], [1, TILE_W]],
                                )
                                eng.dma_start(g[h * ROWS:(h + 1) * ROWS, :], src)

                    if with_compute:
                        m = mpool.tile([128, TILE_W], i16, tag="m")
                        nc.scalar.activation(
                            m[:], ramp[:], AF.Relu, bias=bias1_sb[:, k:k + 1], scale=1.0
                        )
                        t = tpool.tile([128, TILE_W], i16, tag="t")
                        nc.vector.tensor_scalar(
                            t[:], ramp[:], thr2_sb[:, k:k + 1], 1025, OP.is_ge, OP.add
                        )
                        if g is None:
                            g = gpool.tile([128, TILE_W], i16, tag="g")
                        nc.vector.copy_predicated(g[:], m[:], t[:])

                    if with_dma:
                        engs = store_eng(k)
                        if len(engs) == 1:
                            dst = bass.AP(
                                out16,
                                k * TILE_W,
                                [[HALF_SPAN, 2], [TOUT, ROWS], [1, TILE_W]],
                            )
                            engs[0].dma_start(dst, g[:, :])
                        else:
                            for h, eng in enumerate(engs):
                                dst = bass.AP(
                                    out16,
                                    h * HALF_SPAN + k * TILE_W,
                                    [[TOUT, ROWS], [1, TILE_W]],
                                )
                                eng.dma_start(dst, g[h * ROWS:(h + 1) * ROWS, :])

                # Trailing TAIL_W columns at t0 = 2*HALF_SPAN = 32768.
                if mode != "full":
                    continue
                g8 = gpool.tile([ROWS, TAIL_W], i16, tag="g8")
                src = bass.AP(audio16, 2 * HALF_SPAN, [[PADW, ROWS], [1, TAIL_W]])
                nc.sync.dma_start(g8[:, :], src)
                m8 = mpool.tile([ROWS, TAIL_W], i16, tag="m8")
                nc.scalar.activation(
                    m8[:], ramp[0:ROWS, 0:TAIL_W], AF.Relu,
                    bias=bias1_sb[0:ROWS, N_ITERS:N_ITERS + 1], scale=1.0,
                )
                t8 = tpool.tile([ROWS, TAIL_W], i16, tag="t8")
                nc.vector.tensor_scalar(
                    t8[:], ramp[0:ROWS, 0:TAIL_W],
                    thr2_sb[0:ROWS, N_ITERS:N_ITERS + 1], 1025, OP.is_ge, OP.add,
                )
                nc.vector.copy_predicated(g8[:, :], m8[:], t8[:])
                dst = bass.AP(out16, 2 * HALF_SPAN, [[TOUT, ROWS], [1, TAIL_W]])
                nc.scalar.dma_start(dst, g8[:, :])

    nc.finalize()
    return nc


def _get_program(offset: int):
    import os
    cfg = os.environ.get("KERNEL_DMA_CFG", "sync_scalar")
    key = ("prog", offset, cfg)
    if key not in _CACHE:
        _CACHE[key] = _build(offset, dma_cfg=cfg)
    return _CACHE[key]


def _thresholds(lens: np.ndarray, offset: int):
    """Per-(partition, iteration) local thresholds for one core.

    lens: [B_SHARD] int array of this core's audio lengths.
    Returns (thr2 [128, 5] float32, bias1 [128, 5] float32).
    """
    p = np.arange(128)
    h = p // ROWS
    r = p % ROWS
    c = r // B_SHARD
    b = r % B_SHARD
    ln = lens[b]
    t1 = ln + c + offset          # first non-gathered position
    t2 = ln + C                   # first PAD position
    t0 = np.empty((128, N_ITERS + 1), dtype=np.int64)
    for k in range(N_ITERS):
        t0[:, k] = h * HALF_SPAN + k * TILE_W
    t0[:, N_ITERS] = 2 * HALF_SPAN
    thr2 = (t2[:, None] - t0).astype(np.float32)
    bias1 = np.minimum(1 - (t1[:, None] - t0), 8193).astype(np.float32)
    return thr2, bias1


def _make_in_maps(audio, audio_len, offset):
    in_maps = []
    for i in range(N_CORES):
        shard = audio[i * B_SHARD:(i + 1) * B_SHARD]            # [8, 8, T]
        delayed = np.empty((C, B_SHARD, PADW), dtype=np.int16)  # rows (c, b)
        for c in range(C):
            d = c + offset
            delayed[c, :, :d] = SOS
            delayed[c, :, d:d + T] = shard[:, c, :]
            delayed[c, :, d + T:] = 0
        lens = audio_len[i * B_SHARD:(i + 1) * B_SHARD].astype(np.int64)
        thr2, bias1 = _thresholds(lens, offset)
        in_maps.append({
            "audio16": delayed.reshape(ROWS, PADW),
            "thr2": thr2,
            "bias1": bias1,
        })
    return in_maps


def kernel(audio, audio_len, target=0, **_unused):
    global LAST_RESULTS
    _ensure_import_paths()
    from concourse.bass_utils import run_bass_kernel_spmd

    audio = np.asarray(audio)
    audio_len = np.asarray(audio_len)
    offset = 0 if target else 1

    nc = _get_program(offset)
    in_maps = _make_in_maps(audio, audio_len, offset)

    res = run_bass_kernel_spmd(nc, in_maps, list(range(N_CORES)))
    LAST_RESULTS = res

    parts = []
    for i in range(N_CORES):
        o = np.asarray(res.results[i]["out16"]).reshape(C, B_SHARD, TOUT)
        parts.append(o.transpose(1, 0, 2))                     # [8, 8, TOUT]
    out = np.concatenate(parts, axis=0).astype(audio.dtype)
    return out, (audio_len + C).astype(audio_len.dtype)


# revision 14
# speedup vs baseline: 28.0250x; 5.7991x over previous
"""DelayAudio Trainium2 kernel.

Per-channel delay line with SOS/EOS/PAD masking:
  out[b, c, t] = SOS                       for t <  start_c           (start_c = c + offset)
               = audio[b, c, t - start_c]  for start_c <= t < start_c + len_b
               = EOS                       for start_c + len_b <= t < len_b + C
               = PAD                       for t >= len_b + C
Second output: audio_len + C.

Strategy: pure data-parallel over batch (8 examples per NeuronCore, 8 cores).
On-device dtype is int16 (values fit in [0, 1026]), which halves HBM traffic
vs int32.  The host uploads each core's shard in "delayed layout": rows are
(channel, batch) = 64 rows of width T+16, where row r already carries its
per-channel shift and SOS head (P[r, i] = SOS for i < start_c, else
audio[r, i - start_c]).  That choice of upload layout makes every device-side
load/store ONE dense 3D affine DMA over all 128 partitions.

The dynamic (audio_len-dependent) masking runs on device:
  mask = Relu(ramp + (1 - t1loc))     on ScalarE  (nonzero <=> t >= len + start_c)
  tail = (ramp >= t2loc) + 1025      on VectorE  (1025=EOS before len+C, 1026=PAD)
  copy_predicated(G, mask, tail)     on VectorE
with per-(row, tile) local thresholds precomputed on the host from audio_len
into tiny [128, 5] f32 tensors.  Loads issue on the SP HWDGE ring, stores on
the ACT HWDGE ring so the two directions pipeline independently.
"""

import numpy as np

_BASS_PATHS = ["/opt/trn_rl_repo", "/root/.axon_site/_ro/trn_rl_repo"]


def _ensure_import_paths():
    import sys

    try:
        import concourse.bass  # noqa: F401

        return
    except ImportError:
        pass
    for p in _BASS_PATHS:
        if p not in sys.path:
            sys.path.insert(0, p)
    import concourse.bass  # noqa: F401


# Problem constants (hardcoded per harness contract).
B = 64
C = 8
T = 32768
TOUT = T + C
N_CORES = 8
B_SHARD = B // N_CORES  # 8 examples per core
ROWS = B_SHARD * C      # 64 rows per core, ordered (c, b)
SOS = 1024
EOS = 1025
PAD = 1026

PADW = T + 16           # padded width of the host-uploaded delayed layout
TILE_W = 4096
N_ITERS = 4                   # main loop iterations; each covers 2 tiles (halves)
HALF_SPAN = N_ITERS * TILE_W  # 16384; half h covers [h*16384, (h+1)*16384)
TAIL_W = TOUT - 2 * HALF_SPAN  # 8 trailing columns

_CACHE: dict = {}
LAST_RESULTS = None  # BassKernelResults of the most recent run (for test harness)


def _build(offset: int, repeats: int = 1, mode: str = "full", dma_cfg: str = "sync_scalar"):
    """Build + finalize the (single, shared across cores) Bass program.

    repeats > 1 emits the whole pipeline body that many times (same I/O) —
    used only for timing (per-execution time from deltas between variants).
    mode: "full", "dma" (no compute), or "compute" (no audio/output DMAs).
    """
    import concourse.bass as bass
    import concourse.tile as tile
    from concourse import bacc, mybir

    AF = mybir.ActivationFunctionType
    OP = mybir.AluOpType
    i16 = mybir.dt.int16
    f32 = mybir.dt.float32

    nc = bacc.Bacc()
    audio16 = nc.declare_dram_parameter("audio16", [ROWS, PADW], i16, isOutput=False)
    thr2 = nc.declare_dram_parameter("thr2", [128, N_ITERS + 1], f32, isOutput=False)
    bias1 = nc.declare_dram_parameter("bias1", [128, N_ITERS + 1], f32, isOutput=False)
    out16 = nc.declare_dram_parameter("out16", [ROWS, TOUT], i16, isOutput=True)

    with_dma = mode in ("full", "dma")
    with_compute = mode in ("full", "compute")

    ALL3 = [nc.sync, nc.scalar, nc.gpsimd]

    def load_eng(k):
        return {
            "sync_scalar": [nc.sync],
            "sync_only": [nc.sync],
            "swdge": [nc.gpsimd],
            "split4": [nc.sync, nc.scalar],
            "alt": [ALL3[k % 2]],
            "tri": [ALL3[k % 3]],
            "rr64": [ALL3[(2 * k) % 3], ALL3[(2 * k + 1) % 3]],
            "hw64": [nc.sync, nc.sync],
            "swdge64": [nc.gpsimd, nc.gpsimd],
            "split4r": [nc.gpsimd, nc.gpsimd],
        }[dma_cfg]

    def store_eng(k):
        return {
            "sync_scalar": [nc.scalar],
            "sync_only": [nc.sync],
            "swdge": [nc.gpsimd],
            "split4": [nc.gpsimd, nc.gpsimd],
            "alt": [ALL3[(k + 1) % 2]],
            "tri": [ALL3[(k + 1) % 3]],
            "rr64": [ALL3[(2 * k + 2) % 3], ALL3[(2 * k) % 3]],
            "hw64": [nc.scalar, nc.scalar],
            "swdge64": [nc.gpsimd, nc.gpsimd],
            "split4r": [nc.sync, nc.scalar],
        }[dma_cfg]

    with tile.TileContext(nc) as tc:
        with (
            tc.tile_pool(name="consts", bufs=1) as consts,
            tc.tile_pool(name="gpool", bufs=4) as gpool,
            tc.tile_pool(name="mpool", bufs=3) as mpool,
            tc.tile_pool(name="tpool", bufs=3) as tpool,
        ):
            ramp = consts.tile([128, TILE_W], i16)
            nc.gpsimd.iota(ramp[:], pattern=[[1, TILE_W]], base=0, channel_multiplier=0)
            thr2_sb = consts.tile([128, N_ITERS + 1], f32)
            nc.sync.dma_start(thr2_sb[:], thr2[:])
            bias1_sb = consts.tile([128, N_ITERS + 1], f32)
            nc.sync.dma_start(bias1_sb[:], bias1[:])

            for _rep in range(repeats):
                for k in range(N_ITERS):
                    g = None
                    if with_dma:
                        g = gpool.tile([128, TILE_W], i16, tag="g")
                        engs = load_eng(k)
                        if len(engs) == 1:
                            src = bass.AP(
                                audio16,
                                k * TILE_W,
                                [[HALF_SPAN, 2], [PADW, ROWS], [1, TILE_W]],
                            )
                            engs[0].dma_start(g[:, :], src)
                        else:
                            for h, eng in enumerate(engs):
                                src = bass.AP(
                                    audio16,
                                    h * HALF_SPAN + k * TILE_W,
                                    [[PADW, ROWS], [1, TILE_W]],
                                )
                                eng.dma_start(g[h * ROWS:(h + 1) * ROWS, :], src)

                    if with_compute:
                        m = mpool.tile([128, TILE_W], i16, tag="m")
                        nc.scalar.activation(
                            m[:], ramp[:], AF.Relu, bias=bias1_sb[:, k:k + 1], scale=1.0
                        )
                        t = tpool.tile([128, TILE_W], i16, tag="t")
                        nc.vector.tensor_scalar(
                            t[:], ramp[:], thr2_sb[:, k:k + 1], 1025, OP.is_ge, OP.add
                        )
                        if g is None:
                            g = gpool.tile([128, TILE_W], i16, tag="g")
                        nc.vector.copy_predicated(g[:], m[:], t[:])

                    if with_dma:
                        engs = store_eng(k)
                        if len(engs) == 1:
                            dst = bass.AP(
                                out16,
                                k * TILE_W,
                                [[HALF_SPAN, 2], [TOUT, ROWS], [1, TILE_W]],
                            )
                            engs[0].dma_start(dst, g[:, :])
                        else:
                            for h, eng in enumerate(engs):
                                dst = bass.AP(
                                    out16,
                                    h * HALF_SPAN + k * TILE_W,
                                    [[TOUT, ROWS], [1, TILE_W]],
                                )
                                eng.dma_start(dst, g[h * ROWS:(h + 1) * ROWS, :])

                # Trailing TAIL_W columns at t0 = 2*HALF_SPAN = 32768.
                if mode != "full":
                    continue
                g8 = gpool.tile([ROWS, TAIL_W], i16, tag="g8")
                src = bass.AP(audio16, 2 * HALF_SPAN, [[PADW, ROWS], [1, TAIL_W]])
                nc.sync.dma_start(g8[:, :], src)
                m8 = mpool.tile([ROWS, TAIL_W], i16, tag="m8")
                nc.scalar.activation(
                    m8[:], ramp[0:ROWS, 0:TAIL_W], AF.Relu,
                    bias=bias1_sb[0:ROWS, N_ITERS:N_ITERS + 1], scale=1.0,
                )
                t8 = tpool.tile([ROWS, TAIL_W], i16, tag="t8")
                nc.vector.tensor_scalar(
                    t8[:], ramp[0:ROWS, 0:TAIL_W],
                    thr2_sb[0:ROWS, N_ITERS:N_ITERS + 1], 1025, OP.is_ge, OP.add,
                )
                nc.vector.copy_predicated(g8[:, :], m8[:], t8[:])
                dst = bass.AP(out16, 2 * HALF_SPAN, [[TOUT, ROWS], [1, TAIL_W]])
                nc.scalar.dma_start(dst, g8[:, :])

    nc.finalize()
    return nc


def _get_program(offset: int):
    import os
    cfg = os.environ.get("KERNEL_DMA_CFG", "sync_scalar")
    key = ("prog", offset, cfg)
    if key not in _CACHE:
        _CACHE[key] = _build(offset, dma_cfg=cfg)
    return _CACHE[key]


def _thresholds(lens: np.ndarray, offset: int):
    """Per-(partition, iteration) local thresholds for one core.

    lens: [B_SHARD] int array of this core's audio lengths.
    Returns (thr2 [128, 5] float32, bias1 [128, 5] float32).
    """
    p = np.arange(128)
    h = p // ROWS
    r = p % ROWS
    c = r // B_SHARD
    b = r % B_SHARD
    ln = lens[b]
    t1 = ln + c + offset          # first non-gathered position
    t2 = ln + C                   # first PAD position
    t0 = np.empty((128, N_ITERS + 1), dtype=np.int64)
    for k in range(N_ITERS):
        t0[:, k] = h * HALF_SPAN + k * TILE_W
    t0[:, N_ITERS] = 2 * HALF_SPAN
    thr2 = (t2[:, None] - t0).astype(np.float32)
    bias1 = np.minimum(1 - (t1[:, None] - t0), 8193).astype(np.float32)
    return thr2, bias1


def _make_in_maps(audio, audio_len, offset):
    in_maps = []
    for i in range(N_CORES):
        shard = audio[i * B_SHARD:(i + 1) * B_SHARD]            # [8, 8, T]
        delayed = np.empty((C, B_SHARD, PADW), dtype=np.int16)  # rows (c, b)
        for c in range(C):
            d = c + offset
            delayed[c, :, :d] = SOS
            delayed[c, :, d:d + T] = shard[:, c, :]
            delayed[c, :, d + T:] = 0
        lens = audio_len[i * B_SHARD:(i + 1) * B_SHARD].astype(np.int64)
        thr2, bias1 = _thresholds(lens, offset)
        in_maps.append({
            "audio16": delayed.reshape(ROWS, PADW),
            "thr2": thr2,
            "bias1": bias1,
        })
    return in_maps


def kernel(audio, audio_len, target=0, **_unused):
    global LAST_RESULTS
    _ensure_import_paths()
    from concourse.bass_utils import run_bass_kernel_spmd

    audio = np.asarray(audio)
    audio_len = np.asarray(audio_len)
    offset = 0 if target else 1

    nc = _get_program(offset)
    in_maps = _make_in_maps(audio, audio_len, offset)

    res = run_bass_kernel_spmd(nc, in_maps, list(range(N_CORES)))
    LAST_RESULTS = res

    parts = []
    for i in range(N_CORES):
        o = np.asarray(res.results[i]["out16"]).reshape(C, B_SHARD, TOUT)
        parts.append(o.transpose(1, 0, 2))                     # [8, 8, TOUT]
    out = np.concatenate(parts, axis=0).astype(audio.dtype)
    return out, (audio_len + C).astype(audio_len.dtype)
